# revision 1
# baseline (speedup 1.0000x reference)
"""Trainium2 Bass kernel for nn_Conv_MS_MSA (spectral multi-head self-attention).

Reference computation (per batch):
  qkv = dw3x3_depthwise(conv1x1(x))          # 256 -> 768 ch, then per-ch 3x3
  q, k, v = split(qkv); v_out = v
  per head (8 heads x 32 d): L2-normalize q,k rows over the 65536 pixels,
  attn = softmax(k_norm @ q_norm^T * rescale), out = attn @ v
  out_c = conv3x3_dense(out, w_proj)         # 256 -> 256 ch

Sharding: spatial bands. Core i owns image rows [32i, 32i+32) of BOTH batches,
with halo rows for the two 3x3 convs. The only global coupling is the per-head
32x32 Gram matrices and q/k row norms -- reduced with one small per-batch
AllReduce that overlaps the other batch's compute.

Layouts: channels on SBUF partitions, pixels on the free dim; the whole
datapath runs in bf16 (inputs converted host-side), psum accumulation fp32.
Depthwise taps are split across PE (diagonal matmuls), DVE (4x-mode
tensor_scalar_mul + 2x tensor_tensor add) and Act (scaled copies); Pool does
PSUM evacuation.
"""

import sys

if "/opt/trn_rl_repo" not in sys.path:
    sys.path.insert(0, "/opt/trn_rl_repo")

import numpy as np
import ml_dtypes

import concourse.bass as bass
import concourse.tile as tile
from concourse import bacc, mybir
from concourse import bass_utils

# ---------------------------------------------------------------- problem dims
B = 2
C = 256
H = 256
W = 256
HEADS = 8
N_CORES = 8
ROWS = H // N_CORES          # 32 owned rows per core
VB = ROWS + 2                # 34 v/out band rows (1-row halo each side)
XB = ROWS + 4                # 36 x/qkv band rows (2-row halo each side)
CT = C // 128                # 2 channel tiles of 128 per 256-ch tensor
QKCT = 4                     # q,k channel tiles (512 ch)
EPS = 1e-12

fp32 = mybir.dt.float32
bf16 = mybir.dt.bfloat16

# tap assignment (tap = dy*3+dx) for the q/k depthwise. PE taps accumulate
# in psum; Pool copies that psum into qk (the init); other taps add on top.
QK_PE_TAPS = [0, 2, 5, 7]    # diagonal-matmul taps on the PE
QK_FOLD_TAP = 3              # DVE stt: qk = win*s + psum(PE taps)
QK_DVE_TAPS = [6, 8]         # DVE: ts_mul (4x) + tt add (2x)
QK_POOL_TAPS = [1, 4]        # Pool scaled copy (SBUF only) + DVE tt add
# v depthwise: no PE taps (PE is proj-bound in that phase)
V_INIT_TAP = 0               # DVE ts_mul direct into v band
V_DVE_TAPS = [2, 3, 6]
V_POOL_TAPS = [1, 4, 8]
V_ACT_TAPS = [5, 7]

CHUNK = 8                    # QK-pass rows per chunk
V_CHUNKS = [(0, 8), (8, 8), (16, 8), (24, 8), (32, 2)]   # over VB

Alu = mybir.AluOpType
Act = mybir.ActivationFunctionType

_CONST_POOL = None


def _single(tc, shape, dtype, name):
    return _CONST_POOL.tile(shape, dtype, tag=name, name=name)


def _four_row_groups(total):
    out = []
    s = 0
    while s < total:
        out.append((s, min(4, total - s)))
        s += 4
    return out


def build_program():
    nc = bacc.Bacc(
        "TRN2", target_bir_lowering=False, debug=False, num_devices=N_CORES
    )

    # ------------------------------------------------------------- DRAM I/O
    x_d = nc.dram_tensor("x", [B, CT, 128, XB, 256], bf16, kind="ExternalInput")
    wq_d = nc.dram_tensor("wq", [128, CT, 768], bf16, kind="ExternalInput")
    wdw_d = nc.dram_tensor("wdw", [128, 6, 9], fp32, kind="ExternalInput")
    wp_d = nc.dram_tensor("wp", [128, CT, 9, 256], bf16, kind="ExternalInput")
    identb_d = nc.dram_tensor("identb", [128, 128], bf16, kind="ExternalInput")
    resc_d = nc.dram_tensor("resc", [128, CT], fp32, kind="ExternalInput")
    hmask_d = nc.dram_tensor("hmask", [128, 2], fp32, kind="ExternalInput")

    vband_d = nc.dram_tensor(
        "vband", [B, CT, 128, VB, 256], bf16, kind="ExternalOutput"
    )
    outc_d = nc.dram_tensor(
        "outc", [B, CT, 128, ROWS, 256], bf16, kind="ExternalOutput"
    )

    with tile.TileContext(nc) as tc:
        global _CONST_POOL
        with tc.tile_pool(name="consts", bufs=1) as cpool:
            _CONST_POOL = cpool
            _build(nc, tc, x_d, wq_d, wdw_d, wp_d, identb_d, resc_d, hmask_d,
                   vband_d, outc_d)
            _CONST_POOL = None
    nc.compile()
    return nc


def _build(nc, tc, x_d, wq_d, wdw_d, wp_d, identb_d, resc_d, hmask_d,
           vband_d, outc_d):
    # ------------------------------------------------------ constants in SBUF
    wq = _single(tc, [128, CT, 768], bf16, name="wq_sb")
    wdw = _single(tc, [128, 6, 9], fp32, name="wdw_sb")
    wp = _single(tc, [128, CT, 9, 256], bf16, name="wp_sb")
    identb = _single(tc, [128, 128], bf16, name="identb_sb")
    resc = _single(tc, [128, CT], fp32, name="resc_sb")
    hmask = _single(tc, [128, 2], fp32, name="hmask_sb")
    nc.sync.dma_start(wq[:], wq_d[:, :, :])
    nc.sync.dma_start(wdw[:], wdw_d[:, :, :])
    nc.sync.dma_start(wp[:], wp_d[:, :, :, :])
    nc.sync.dma_start(identb[:], identb_d[:, :])
    nc.sync.dma_start(resc[:], resc_d[:, :])
    nc.sync.dma_start(hmask[:], hmask_d[:, :])

    # diagonal weight matrices for the PE depthwise taps (q/k tiles only)
    diags = {}
    for t in range(QKCT):
        for tp in QK_PE_TAPS:
            d = _single(tc, [128, 128], bf16, name=f"diagb_{t}_{tp}")
            nc.vector.tensor_scalar_mul(
                d[:], identb[:], wdw[:, t, tp : tp + 1]
            )
            diags[(t, tp)] = d

    # per-batch stats: cols [0,64) = per-head gram diag blocks (g*32 + e),
    # cols [64,68) = sumsq per q/k channel tile
    stats = _single(tc, [128, B, 68], fp32, name="stats_sb")
    stats2 = _single(tc, [128, B, 68], fp32, name="stats2_sb")
    nc.gpsimd.memset(stats[:], 0.0)

    bd = {}
    wpa_b = {}

    with (
        tc.tile_pool(name="xp", bufs=2) as p_x,
        tc.tile_pool(name="qkvp", bufs=3) as p_qkv,
        tc.tile_pool(name="qkp", bufs=8) as p_qk,
        tc.tile_pool(name="ascrp", bufs=4) as p_ascr,
        tc.tile_pool(name="dscrp", bufs=3) as p_dscr,
        tc.tile_pool(name="sqa", bufs=2) as p_sqa,
        tc.tile_pool(name="qtp", bufs=2) as p_qt,
        tc.tile_pool(name="vbp", bufs=2) as p_vb,
        tc.tile_pool(name="wpap", bufs=2) as p_wpa,
        tc.tile_pool(name="ocp", bufs=2) as p_oc,
        tc.tile_pool(name="smx", bufs=1) as p_sm,
        tc.tile_pool(name="psa", bufs=2, space="PSUM") as ps_a,
        tc.tile_pool(name="pstr", bufs=2, space="PSUM") as ps_tr,
        tc.tile_pool(name="psgp", bufs=2, space="PSUM") as ps_gp,
        tc.tile_pool(name="ardram", bufs=2, space="DRAM") as p_ar,
    ):
        def win(qkv_t, tp, L):
            dy, dx = divmod(tp, 3)
            return qkv_t[:, dy : dy + L, dx : dx + 256]

        def sc(t, tp):
            return wdw[:, t, tp : tp + 1]

        # ==================================================== QK pass (per b)
        def qk_pass(b):
            g_ps = ps_gp.tile([128, 2, 256], fp32, tag="psgp", name="g_ps")
            n_chunks = ROWS // CHUNK
            chunk_tiles = {}

            def trgram(cj):
                tiles = chunk_tiles.pop(cj)
                nblk = (CHUNK * 256) // 128
                for blk in range(nblk):
                    r, cb = blk // 2, (blk % 2) * 128
                    pst = ps_tr.tile([128, 512], bf16, tag="pst", name="pst")
                    for half in range(2):
                        nc.tensor.matmul(
                            pst[:, 128 * half : 128 * half + 128],
                            tiles[half][:, r, cb : cb + 128],
                            identb[:],
                            is_transpose=True,
                            skip_group_check=True,
                        )
                        nc.tensor.matmul(
                            pst[:, 256 + 128 * half : 384 + 128 * half],
                            tiles[2 + half][:, r, cb : cb + 128],
                            identb[:],
                            is_transpose=True,
                            skip_group_check=True,
                        )
                    qkt = p_qt.tile([128, 512], bf16, tag="qkt", name="qkt")
                    nc.scalar.copy(qkt[:], pst[:])
                    first = cj == 0 and blk == 0
                    last = cj == n_chunks - 1 and blk == nblk - 1
                    for g in range(2):
                        nc.tensor.matmul(
                            g_ps[:, g, :],
                            qkt[:, 256 + g * 128 : 384 + g * 128],
                            qkt[:, 0:256],
                            start=first,
                            stop=last,
                            skip_group_check=True,
                        )

            for ci in range(n_chunks):
                s = 1 + ci * CHUNK
                L = CHUNK
                x_qc = p_x.tile([128, CT, L + 2, 256], bf16, tag="x", name="x_qc")
                for kt in range(CT):
                    nc.sync.dma_start(
                        x_qc[:, kt], x_d[b, kt][:, s : s + L + 2, :]
                    )
                qk_tiles = []
                for t in range(QKCT):
                    qkv_t = p_qkv.tile([128, L + 2, 258], bf16, tag="qkv",
                                       name="qkv_t")
                    nc.gpsimd.memset(qkv_t[:, :, 0], 0.0)
                    nc.gpsimd.memset(qkv_t[:, :, 257], 0.0)
                    # conv1x1 into 4-row psum tiles, Pool evacuates
                    for r0, rn in _four_row_groups(L + 2):
                        ps = ps_a.tile([128, 4, 256], fp32, tag="psA",
                                       name="ps_c")
                        for g2 in range(rn // 2):
                            r = r0 + 2 * g2
                            for kt in range(CT):
                                nc.tensor.matmul(
                                    ps[:, 2 * g2 : 2 * g2 + 2, :],
                                    wq[:, kt, t * 128 : (t + 1) * 128],
                                    x_qc[:, kt, r : r + 2, :],
                                    start=(kt == 0),
                                    stop=(kt == CT - 1),
                                    skip_group_check=True,
                                )
                        nc.scalar.copy(
                            qkv_t[:, r0 : r0 + rn, 1:257], ps[:, :rn, :]
                        )
                    qk_t = p_qk.tile([128, L, 256], bf16, tag="qk", name="qk_t")
                    # PE taps -> psum (4-row tiles, 2 groups each); a DVE
                    # stt folds the psum together with tap QK_FOLD_TAP
                    dyi, dxi = divmod(QK_FOLD_TAP, 3)
                    for half in range(L // 4):
                        psd = ps_a.tile([128, 4, 256], fp32, tag="psA",
                                        name="ps_d")
                        for g2 in range(2):
                            r = 4 * half + 2 * g2
                            for j, tp in enumerate(QK_PE_TAPS):
                                dy, dx = divmod(tp, 3)
                                nc.tensor.matmul(
                                    psd[:, 2 * g2 : 2 * g2 + 2, :],
                                    diags[(t, tp)][:],
                                    qkv_t[:, r + dy : r + dy + 2,
                                          dx : dx + 256],
                                    start=(j == 0),
                                    stop=(j == len(QK_PE_TAPS) - 1),
                                    skip_group_check=True,
                                )
                        nc.vector.scalar_tensor_tensor(
                            qk_t[:, 4 * half : 4 * half + 4, :],
                            qkv_t[:, 4 * half + dyi : 4 * half + dyi + 4,
                                  dxi : dxi + 256],
                            sc(t, QK_FOLD_TAP),
                            psd[:, :, :],
                            op0=Alu.mult, op1=Alu.add,
                        )
                    # Pool taps: scaled copy to scratch (adds folded on DVE)
                    act_scr = []
                    for tp in QK_POOL_TAPS:
                        scr = p_ascr.tile([128, L, 256], bf16, tag="ascr",
                                          name="scr_a")
                        nc.gpsimd.tensor_scalar_mul(
                            scr[:], win(qkv_t, tp, L), sc(t, tp)
                        )
                        act_scr.append(scr)
                    # DVE taps: 4x-mode mul to scratch, then 2x add
                    for tp in QK_DVE_TAPS:
                        scr = p_dscr.tile([128, L, 256], bf16, tag="dscr",
                                          name="scr_d")
                        nc.vector.tensor_scalar_mul(
                            scr[:], win(qkv_t, tp, L), sc(t, tp)
                        )
                        nc.vector.tensor_tensor(
                            qk_t[:], qk_t[:], scr[:], op=Alu.add
                        )
                    for scr in act_scr:
                        nc.vector.tensor_tensor(
                            qk_t[:], qk_t[:], scr[:], op=Alu.add
                        )
                    qk_tiles.append(qk_t)

                    # sumsq of this chunk -> stats col 64 + t
                    scr_sq = p_dscr.tile([128, L, 256], bf16, tag="dscr",
                                         name="scr_sq")
                    sq = p_sqa.tile([128, 1], fp32, tag="sqa", name="sq")
                    nc.scalar.activation(
                        scr_sq[:], qk_t[:], Act.Square, accum_out=sq[:]
                    )
                    nc.vector.tensor_tensor(
                        stats[:, b, 64 + t : 65 + t],
                        stats[:, b, 64 + t : 65 + t],
                        sq[:],
                        op=Alu.add,
                    )

                chunk_tiles[ci] = qk_tiles
                # transposes + Gram lag one chunk so the PE can run the next
                # chunk's conv while the DVE finishes this chunk's taps
                if ci > 0:
                    trgram(ci - 1)
            trgram(n_chunks - 1)

            # extract per-head diagonal 32x32 blocks of the Gram
            for g in range(2):
                for i in range(4):
                    h = 4 * g + i
                    nc.vector.tensor_copy(
                        stats[32 * i : 32 * i + 32, b,
                              g * 32 : g * 32 + 32],
                        g_ps[32 * i : 32 * i + 32, g,
                             32 * h : 32 * h + 32],
                    )

        # ================================================== AllReduce kick
        def ar_kick():
            ar_in = p_ar.tile([128, B, 68], fp32, tag="arin", name="ar_in")
            ar_out = p_ar.tile([128, B, 68], fp32, tag="arout",
                               addr_space="Shared", name="ar_out")
            nc.sync.dma_start(ar_in[:], stats[:])
            nc.gpsimd.collective_compute(
                "AllReduce",
                Alu.add,
                replica_groups=[list(range(N_CORES))],
                ins=[ar_in[:].opt()],
                outs=[ar_out[:].opt()],
            )
            nc.sync.dma_start(stats2[:], ar_out[:])

        # ================================================== softmax (per b)
        def softmax(b):
            st = stats2[:, b]
            # rsq[:, idx] = 1 / max(sqrt(sumsq), eps), idx = qk*2 + g
            rsq = p_sm.tile([128, 4], fp32, tag="rsq", name="rsq")
            nc.scalar.activation(rsq[:], st[:, 64:68], Act.Sqrt)
            nc.vector.tensor_scalar_max(rsq[:], rsq[:], EPS)
            nc.vector.reciprocal(rsq[:], rsq[:])
            for g in range(2):
                kcol = 2 + g
                qcol = g
                ksc = p_sm.tile([128, 1], fp32, tag="ksc", name="ksc")
                nc.vector.tensor_tensor(
                    ksc[:], rsq[:, kcol : kcol + 1], resc[:, g : g + 1],
                    op=Alu.mult,
                )
                t1 = p_sm.tile([128, 32], fp32, tag="t1", name="t1")
                graw = st[:, g * 32 : g * 32 + 32]
                nc.vector.tensor_scalar_mul(t1[:], graw, ksc[:])
                # M[p, j] = rsq_q[32*(p//32) + j]: broadcast + block-transpose
                a2 = p_sm.tile([128, 32], fp32, tag="a2", name="a2")
                nc.vector.tensor_scalar(
                    a2[:], t1[:], 0.0, rsq[:, qcol : qcol + 1],
                    op0=Alu.mult, op1=Alu.add,
                )
                m = p_sm.tile([128, 32], fp32, tag="m", name="m")
                nc.vector.transpose(m[:], a2[:])
                nc.vector.tensor_tensor(t1[:], t1[:], m[:], op=Alu.mult)
                # softmax over the free (e) dim
                mx = p_sm.tile([128, 1], fp32, tag="mx", name="mx")
                nc.vector.tensor_reduce(
                    mx[:], t1[:], mybir.AxisListType.X, Alu.max
                )
                nc.vector.tensor_scalar_sub(t1[:], t1[:], mx[:])
                ex = p_sm.tile([128, 32], fp32, tag="ex", name="ex")
                nc.scalar.activation(ex[:], t1[:], Act.Exp)
                sm = p_sm.tile([128, 1], fp32, tag="sm", name="sm")
                nc.vector.tensor_reduce(
                    sm[:], ex[:], mybir.AxisListType.X, Alu.add
                )
                nc.vector.reciprocal(sm[:], sm[:])
                at = p_sm.tile([128, 32], fp32, tag="at", name="at")
                nc.vector.tensor_scalar_mul(at[:], ex[:], sm[:])
                # block-diagonal A (lhsT for the Wp_tap @ A fusion)
                bdt = _single(tc, [128, 128], bf16, name=f"bd_{b}_{g}")
                nc.gpsimd.memset(bdt[:], 0.0)
                for i in range(4):
                    nc.vector.tensor_copy(
                        bdt[32 * i : 32 * i + 32, 32 * i : 32 * i + 32],
                        at[32 * i : 32 * i + 32, :],
                    )
                bd[(b, g)] = bdt
            # fused proj weights: wpa[j, o] = sum_i Wp[o, i, tap] * A[i, j]
            wpa = p_wpa.tile([128, CT, 9, 256], bf16, tag="wpa", name="wpa")
            for g in range(CT):
                for tp2 in range(0, 9, 2):
                    npair = min(2, 9 - tp2)
                    psw = ps_gp.tile([128, 2, 256], fp32, tag="psgp",
                                     name="ps_w")
                    for jj in range(npair):
                        nc.tensor.matmul(
                            psw[:, jj, :],
                            bd[(b, g)][:],
                            wp[:, g, tp2 + jj, :],
                            start=True,
                            stop=True,
                            skip_group_check=True,
                        )
                    nc.scalar.copy(
                        wpa[:, g, tp2 : tp2 + npair, :], psw[:, :npair, :]
                    )
            wpa_b[b] = wpa

        # ====================================== V + fused attn-proj (per b)
        v_band_b = {}

        def v_chunk(b, ci, evac_on_act=False):
            if ci == 0:
                v_band_b[b] = p_vb.tile([128, CT, VB, 258], bf16, tag="vband",
                                        name="v_band")
                for t in range(CT):
                    nc.vector.memset(v_band_b[b][:, t, :, 0], 0.0)
                    nc.vector.memset(v_band_b[b][:, t, :, 257], 0.0)
            v_band = v_band_b[b]
            c0, Lv = V_CHUNKS[ci]
            LX = Lv + 2
            x_c = p_x.tile([128, CT, LX, 256], bf16, tag="x", name="x_c")
            for kt in range(CT):
                nc.sync.dma_start(
                    x_c[:, kt], x_d[b, kt][:, c0 : c0 + LX, :]
                )
            for t in range(CT):
                qkv_t = p_qkv.tile([128, LX, 258], bf16, tag="qkv",
                                   name="qkv_v")
                nc.vector.memset(qkv_t[:, :, 0], 0.0)
                nc.vector.memset(qkv_t[:, :, 257], 0.0)
                for r0, rn in _four_row_groups(LX):
                    ps = ps_a.tile([128, 4, 256], fp32, tag="psA",
                                   name="ps_cv")
                    for g2 in range(rn // 2):
                        r = r0 + 2 * g2
                        for kt in range(CT):
                            nc.tensor.matmul(
                                ps[:, 2 * g2 : 2 * g2 + 2, :],
                                wq[:, kt, (QKCT + t) * 128 :][:, :128],
                                x_c[:, kt, r : r + 2, :],
                                start=(kt == 0),
                                stop=(kt == CT - 1),
                                skip_group_check=True,
                            )
                    nc.scalar.copy(
                        qkv_t[:, r0 : r0 + rn, 1:257], ps[:, :rn, :]
                    )
                v3 = v_band[:, t, c0 : c0 + Lv, 1:257]
                tv = QKCT + t
                nc.vector.tensor_scalar_mul(
                    v3, win(qkv_t, V_INIT_TAP, Lv), sc(tv, V_INIT_TAP)
                )
                act_scr = []
                for tp in V_POOL_TAPS:
                    scr = p_ascr.tile([128, Lv, 256], bf16, tag="ascr",
                                      name="scr_vp")
                    if evac_on_act:
                        nc.scalar.mul(scr[:], win(qkv_t, tp, Lv), sc(tv, tp))
                    else:
                        nc.gpsimd.tensor_scalar_mul(
                            scr[:], win(qkv_t, tp, Lv), sc(tv, tp)
                        )
                    act_scr.append(scr)
                for tp in V_ACT_TAPS:
                    scr = p_ascr.tile([128, Lv, 256], bf16, tag="ascr",
                                      name="scr_va")
                    nc.scalar.mul(scr[:], win(qkv_t, tp, Lv), sc(tv, tp))
                    act_scr.append(scr)
                for tp in V_DVE_TAPS:
                    scr = p_dscr.tile([128, Lv, 256], bf16, tag="dscr",
                                      name="scr_vd")
                    nc.vector.tensor_scalar_mul(
                        scr[:], win(qkv_t, tp, Lv), sc(tv, tp)
                    )
                    nc.vector.tensor_tensor(v3, v3, scr[:], op=Alu.add)
                for scr in act_scr:
                    nc.vector.tensor_tensor(v3, v3, scr[:], op=Alu.add)
                # halo masking at image edges
                if c0 == 0:
                    nc.vector.tensor_scalar_mul(
                        v_band[:, t, 0, 1:257], v_band[:, t, 0, 1:257],
                        hmask[:, 0:1]
                    )
                if c0 + Lv == VB:
                    nc.vector.tensor_scalar_mul(
                        v_band[:, t, VB - 1, 1:257],
                        v_band[:, t, VB - 1, 1:257],
                        hmask[:, 1:2]
                    )
            if c0 + Lv == VB:
                # vband output: owned band rows [1, 33)
                for t in range(CT):
                    nc.sync.dma_start(
                        vband_d[b, t][:, 1:33, :], v_band[:, t, 1:33, 1:257]
                    )

        def v_proj(b, grp):
            v_band = v_band_b[b]
            for mt in range(CT):
                for j in range(4):
                    n = grp * 4 + j
                    ps = ps_gp.tile([128, 2, 256], fp32, tag="psgp",
                                    name="ps_pj")
                    idx = 0
                    for tp in range(9):
                        dy, dx = divmod(tp, 3)
                        for kt in range(CT):
                            nc.tensor.matmul(
                                ps[:],
                                wpa_b[b][:, kt, tp,
                                         mt * 128 : mt * 128 + 128],
                                v_band[:, kt, 2 * n + dy : 2 * n + dy + 2,
                                       dx : dx + 256],
                                start=(idx == 0),
                                stop=(idx == 17),
                                skip_group_check=True,
                            )
                            idx += 1
                    oc = p_oc.tile([128, 2, 256], bf16, tag="oc",
                                   name="oc_t")
                    nc.scalar.copy(oc[:], ps[:])
                    nc.sync.dma_start(
                        outc_d[b, mt][:, 2 * n : 2 * n + 2, :], oc[:]
                    )

        # ------------------------------------------------------ the schedule
        # Single AllReduce after both QK passes; it blocks the Pool queue for
        # ~28us, so the first V chunks of b0 evacuate psum on Act instead.
        # Cross-batch stagger: b0's last proj group is held back until b1's
        # first v chunk is emitted, keeping the PE fed while b1's taps run.
        qk_pass(0)
        qk_pass(1)
        ar_kick()
        v_chunk(0, 0, evac_on_act=True)
        v_chunk(0, 1, evac_on_act=True)
        v_chunk(0, 2)
        softmax(0)
        v_chunk(0, 3)
        v_chunk(0, 4)
        v_proj(0, 0)
        v_proj(0, 1)
        softmax(1)
        v_chunk(1, 0)
        v_proj(0, 2)
        v_chunk(1, 1)
        v_proj(0, 3)
        v_chunk(1, 2)
        v_proj(1, 0)
        v_chunk(1, 3)
        v_proj(1, 1)
        v_chunk(1, 4)
        v_proj(1, 2)
        v_proj(1, 3)



# ------------------------------------------------------------------- host side
_NC_CACHE = None


def _get_program():
    global _NC_CACHE
    if _NC_CACHE is None:
        _NC_CACHE = build_program()
    return _NC_CACHE


def kernel(x_in, w_qkv, w_dw, rescale, w_proj):
    bfl = ml_dtypes.bfloat16
    x_in = np.asarray(x_in, dtype=np.float32)
    w_qkv = np.asarray(w_qkv, dtype=np.float32)
    w_dw = np.asarray(w_dw, dtype=np.float32)
    rescale = np.asarray(rescale, dtype=np.float32)
    w_proj = np.asarray(w_proj, dtype=np.float32)

    # x: NHWC -> NCHW, pad 2 halo rows top/bottom
    xT = np.transpose(x_in, (0, 3, 1, 2))                    # [B, C, H, W]
    xpad = np.zeros((B, C, H + 4, W), bfl)
    xpad[:, :, 2 : H + 2, :] = xT.astype(bfl)

    # weights in device layouts
    wq_h = w_qkv[:, :, 0, 0]                                 # [768, 256]
    wq_l = np.ascontiguousarray(
        wq_h.T.reshape(CT, 128, 768).transpose(1, 0, 2)
    ).astype(bfl)                                            # [128, CT, 768]
    wdw_l = np.ascontiguousarray(
        w_dw[:, 0].reshape(6, 128, 9).transpose(1, 0, 2)
    )                                                        # [128, 6, 9]
    wp_l = np.ascontiguousarray(
        w_proj.transpose(1, 2, 3, 0)                         # [i, 3, 3, o]
        .reshape(C, 9, C)
        .reshape(CT, 128, 9, C)
        .transpose(1, 0, 2, 3)
    ).astype(bfl)                                            # [128, CT, 9, 256]
    identb = np.eye(128, dtype=bfl)
    resc_l = np.empty((128, CT), np.float32)
    r = rescale.reshape(HEADS)
    for g in range(CT):
        resc_l[:, g] = np.repeat(r[4 * g : 4 * g + 4], 32)

    in_maps = []
    for i in range(N_CORES):
        band = np.ascontiguousarray(
            xpad[:, :, 32 * i : 32 * i + XB, :]
        ).reshape(B, CT, 128, XB, 256)
        hm = np.ones((128, 2), np.float32)
        if i == 0:
            hm[:, 0] = 0.0
        if i == N_CORES - 1:
            hm[:, 1] = 0.0
        in_maps.append(
            {
                "x": band,
                "wq": wq_l,
                "wdw": wdw_l,
                "wp": wp_l,
                "identb": identb,
                "resc": resc_l,
                "hmask": hm,
            }
        )

    nc = _get_program()
    res = bass_utils.run_bass_kernel_spmd(
        nc, in_maps, core_ids=list(range(N_CORES))
    )

    v_out = np.empty((B, C, H, W), np.float32)
    outc = np.empty((B, C, H, W), np.float32)
    for i in range(N_CORES):
        vb = res.results[i]["vband"]                 # [B, CT, 128, VB, 256]
        oc = res.results[i]["outc"]                  # [B, CT, 128, ROWS, 256]
        v_out[:, :, 32 * i : 32 * i + 32, :] = (
            vb[:, :, :, 1:33, :].astype(np.float32).reshape(B, C, 32, 256)
        )
        outc[:, :, 32 * i : 32 * i + 32, :] = (
            oc.astype(np.float32).reshape(B, C, 32, 256)
        )

    out_c = np.ascontiguousarray(np.transpose(outc, (0, 2, 3, 1)))
    return (out_c, v_out)

